# revision 1
# baseline (speedup 1.0000x reference)
"""Trainium2 Bass kernel for nn_DZSpecimenClfToy.

Reference computation (per batch item b, B=8, one NeuronCore each):
  1. tv = bilinear_resize(topview[b], (3,64,64) -> (3,4,4))   # fixed 2x2 avg of rows/cols {7,8},{23,24},{39,40},{55,56}
  2. coords = sigmoid(tv.flat @ W1.T + b1).reshape(N,2)       # N=4096
  3. patch top-left tl = coords*2043 (+2-2); all 16 output px of a 4x4
     patch share one bilinear fraction pair -> 5x5 pixel support
  4. out[b] = bilinear_crops.flat @ W2.T + b2                 # [2]

Sharding: data-parallel over batch across 8 cores; weights replicated.

Gather strategy: the toolchain's indirect DMA only supports ONE address per
partition per instruction, so the host uploads the search view in an
overlapped-band layout: 511 bands of 8 rows (stride 4), each stored
[col][row_in_band][ch]. A patch's 5x5x3 support is then one contiguous
111-float run starting at triple index b*16384 + c0*8 + s (b=r0//4,
s=r0%4), and the extraction offsets d*24+(i+di)*3+c are independent of s.
4096 patches = 32 indirect DMAs of [128 partitions x 1 address].
"""
import functools
from contextlib import ExitStack

import numpy as np

import concourse.bass as bass
import concourse.tile as tile
from concourse import bacc, mybir
import concourse.bass_utils as bass_utils
from concourse.bass import IndirectOffsetOnAxis

F32 = mybir.dt.float32
I32 = mybir.dt.int32
ALU = mybir.AluOpType
ACT = mybir.ActivationFunctionType
AX = mybir.AxisListType

B = 8          # batch == number of cores
H = W = 2048   # search view height/width
N = 4096       # patches per item
PS = 4         # patch size
NCLS = 2       # classes
P = 128        # partitions
TPP = N // P   # patches per partition = 32

NBAND = 511            # bands of 8 rows, stride 4: rows 4b..4b+7
BANDTRIP = W * 8       # pixel-triples per band = 16384
SEG = 111              # gathered f32 per patch (covers d*24+(i+di)*3+c <= 110)
SEGP = 128             # SBUF stride per patch segment
MAGIC = 8388608.0      # 2**23


def build_program(num_devices: int, svh: int, svw: int, debug: bool = False):
    pad = float(svh - 1 - PS)  # 2043
    assert svh == H and svw == W, (svh, svw)

    nc = bacc.Bacc("TRN2", target_bir_lowering=False, debug=False,
                   enable_asserts=False, num_devices=num_devices)

    tv = nc.dram_tensor("tv", [3, 64, 64], F32, kind="ExternalInput").ap()
    svb = nc.dram_tensor("svb", [NBAND * BANDTRIP, 3], F32, kind="ExternalInput").ap()
    w1 = nc.dram_tensor("W1", [2 * N, 48], F32, kind="ExternalInput").ap()
    b1 = nc.dram_tensor("b1", [2 * N], F32, kind="ExternalInput").ap()
    w2 = nc.dram_tensor("W2p", [NCLS, N * PS * PS * 3], F32, kind="ExternalInput").ap()
    b2 = nc.dram_tensor("b2", [NCLS], F32, kind="ExternalInput").ap()
    out = nc.dram_tensor("out", [1, NCLS], F32, kind="ExternalOutput").ap()

    dbg = {}
    if debug:
        dbg["s"] = nc.dram_tensor("dbg_s", [P, 2 * TPP], F32, kind="ExternalOutput").ap()
        dbg["idx"] = nc.dram_tensor("dbg_idx", [P, TPP], I32, kind="ExternalOutput").ap()
        dbg["S"] = nc.dram_tensor("dbg_S", [P, TPP * SEGP], F32, kind="ExternalOutput").ap()
        dbg["U"] = nc.dram_tensor("dbg_U", [P, TPP * 48], F32, kind="ExternalOutput").ap()

    with tile.TileContext(nc) as tc:
        with ExitStack() as ctx:
            pool = ctx.enter_context(tc.tile_pool(name="main", bufs=1))

            # ---- input DMAs -------------------------------------------------
            # Topview rows {7,8},{23,24},{39,40},{55,56}: each pair is 128
            # contiguous floats starting at row 7 of each 16-row group.
            A = pool.tile([1, 1536], F32)          # [(c,k), r01*64]
            tv_sel = tv.rearrange("c (k s) w -> c k (s w)", s=16)[:, :, 7 * 64:9 * 64]
            nc.sync.dma_start(A[:].rearrange("p (c k x) -> p c k x", c=3, k=4),
                              tv_sel.unsqueeze(0))

            W1sb = pool.tile([P, 64 * 48], F32)    # row g=p*64+j at [p, j*48:...]
            nc.sync.dma_start(W1sb[:], w1.rearrange("(p j) k -> p (j k)", p=P))

            b1sb = pool.tile([P, 64], F32)
            nc.sync.dma_start(b1sb[:], b1.rearrange("(p j) -> p j", p=P))

            W2sb = pool.tile([P, NCLS * 1536], F32)  # [p, c*1536+x] = W2p[c, p*1536+x]
            nc.sync.dma_start(W2sb[:].rearrange("p (c x) -> p c x", c=NCLS),
                              w2.rearrange("c (p x) -> p c x", p=P))

            b2sb = pool.tile([1, NCLS], F32)
            nc.sync.dma_start(b2sb[:], b2.unsqueeze(0))

            # ---- topview 64x64 -> 4x4 resize, flatten, scale ---------------
            V = pool.tile([1, 768], F32)           # [(c,k), 64] row-pair sums
            A4 = A[:].rearrange("p (ck r w) -> p ck r w", ck=12, r=2)
            nc.vector.tensor_add(V[:].rearrange("p (ck w) -> p ck w", ck=12),
                                 A4[:, :, 0, :], A4[:, :, 1, :])
            F48 = pool.tile([1, 48], F32)
            V4 = V[:].rearrange("p (ck g s) -> p ck g s", ck=12, g=4)
            nc.vector.tensor_add(F48[:].rearrange("p (ck g) -> p ck g", ck=12),
                                 V4[:, :, :, 7], V4[:, :, :, 8])
            flatF = pool.tile([1, 48], F32)
            nc.vector.tensor_scalar_mul(flatF[:], F48[:], 0.25)

            # broadcast flat to all partitions (bounce through DRAM)
            dram_pool = ctx.enter_context(tc.tile_pool(name="dram", bufs=1, space="DRAM"))
            fdram = dram_pool.tile([1, 48], F32)
            nc.sync.dma_start(fdram[:], flatF[:])
            flatb = pool.tile([P, 48], F32)
            nc.sync.dma_start(flatb[:], fdram[:].to_broadcast((P, 48)))

            # ---- coords = sigmoid(W1 @ flat + b1), [128, 64] ---------------
            mul1 = pool.tile([P, 64 * 48], F32)
            nc.vector.tensor_mul(mul1[:].rearrange("p (j k) -> p j k", j=64),
                                 W1sb[:].rearrange("p (j k) -> p j k", j=64),
                                 flatb[:].unsqueeze(1).to_broadcast((P, 64, 48)))
            pre = pool.tile([P, 64], F32)
            nc.vector.reduce_sum(pre[:].unsqueeze(2),
                                 mul1[:].rearrange("p (j k) -> p j k", j=64),
                                 axis=AX.X)
            preb = pool.tile([P, 64], F32)
            nc.vector.tensor_add(preb[:], pre[:], b1sb[:])
            sg = pool.tile([P, 64], F32)
            nc.scalar.activation(sg[:], preb[:], ACT.Sigmoid)
            if debug:
                nc.sync.dma_start(dbg["s"], sg[:])

            # ---- patch top-left corners and fractions ----------------------
            s3 = sg[:].rearrange("p (t two) -> p t two", two=2)

            def floor_to(dst, src, tag):
                """dst = floor(src), src >= 0, via round-to-nearest + correction."""
                rnd = pool.tile([P, TPP], F32, tag=f"rnd{tag}")
                nc.vector.tensor_scalar(rnd[:], src, MAGIC, MAGIC,
                                        op0=ALU.add, op1=ALU.subtract)
                gt = pool.tile([P, TPP], F32, tag=f"gt{tag}")
                nc.vector.tensor_tensor(gt[:], rnd[:], src, op=ALU.is_gt)
                nc.vector.tensor_sub(dst, rnd[:], gt[:])

            def corner(sel):
                xs = pool.tile([P, TPP], F32, tag=f"xs{sel}")
                nc.vector.tensor_scalar(xs[:], s3[:, :, sel], pad, float(PS // 2),
                                        op0=ALU.mult, op1=ALU.add)
                tl = pool.tile([P, TPP], F32, tag=f"tl{sel}")
                nc.vector.tensor_scalar_sub(tl[:], xs[:], float(PS // 2))
                c0 = pool.tile([P, TPP], F32, tag=f"c0{sel}")
                floor_to(c0[:], tl[:], f"c{sel}")
                fr = pool.tile([P, TPP], F32, tag=f"fr{sel}")
                nc.vector.tensor_sub(fr[:], tl[:], c0[:])
                return c0, fr

            r0f, fr = corner(0)   # rows
            c0f, fc = corner(1)   # cols

            # ---- gather index (pixel-triple units, +MAGIC bias) ------------
            # band b = r0//4, s = r0%4, idx = b*16384 + c0*8 + s
            bq = pool.tile([P, TPP], F32)
            nc.vector.tensor_scalar_mul(bq[:], r0f[:], 0.25)
            bf = pool.tile([P, TPP], F32)
            floor_to(bf[:], bq[:], "b")
            sres = pool.tile([P, TPP], F32)        # s = r0 - 4b
            nc.vector.tensor_scalar(sres[:], bf[:], -4.0, None, op0=ALU.mult)
            nc.vector.tensor_add(sres[:], sres[:], r0f[:])
            t1 = pool.tile([P, TPP], F32)
            nc.vector.tensor_scalar(t1[:], bf[:], float(BANDTRIP), MAGIC,
                                    op0=ALU.mult, op1=ALU.add)
            t2 = pool.tile([P, TPP], F32)
            nc.vector.tensor_scalar(t2[:], c0f[:], 8.0, None, op0=ALU.mult)
            nc.vector.tensor_add(t2[:], t2[:], sres[:])
            idxf = pool.tile([P, TPP], F32)
            nc.vector.tensor_add(idxf[:], t1[:], t2[:])
            idxi = pool.tile([P, TPP], I32)
            nc.vector.tensor_single_scalar(idxi[:], idxf[:].bitcast(I32),
                                           0x007FFFFF, op=ALU.bitwise_and)
            if debug:
                nc.sync.dma_start(dbg["idx"], idxi[:])

            # ---- gather: one 111-float run per patch, 32 x [128 x 1] -------
            S = pool.tile([P, TPP * SEGP], F32)
            if debug:
                nc.vector.memset(S[:], 0.0)  # the dbg_S dump reads the padding
            for t in range(TPP):
                nc.gpsimd.indirect_dma_start(
                    out=S[:, t * SEGP: t * SEGP + SEG],
                    out_offset=None,
                    in_=svb,
                    in_offset=IndirectOffsetOnAxis(ap=idxi[:, t:t + 1], axis=0),
                )
            if debug:
                nc.sync.dma_start(dbg["S"], S[:])

            # ---- bilinear combine ------------------------------------------
            # segment layout per patch: elem(d, m, c) at d*24 + m*3 + c,
            # m = i + di (0..4). Row interp over di, col interp over dj.
            Sv = S[:].rearrange("p (t x) -> p t x", t=TPP)

            def seg_view(off):
                # [p, t, d(5 cols, stride 24), 12 = (i,c)] at element offset off
                return Sv[:, :, off:off + 120].rearrange(
                    "p t (d e) -> p t d e", d=5)[:, :, :, 0:12]

            D1 = pool.tile([P, TPP * 60], F32)
            D1v = D1[:].rearrange("p (t d e) -> p t d e", t=TPP, d=5)
            nc.vector.tensor_sub(D1v, seg_view(3), seg_view(0))
            M1 = pool.tile([P, TPP * 60], F32)
            M1v = M1[:].rearrange("p (t d e) -> p t d e", t=TPP, d=5)
            nc.vector.tensor_mul(M1v, D1v,
                                 fr[:].unsqueeze(2).unsqueeze(3).to_broadcast((P, TPP, 5, 12)))
            T = pool.tile([P, TPP * 60], F32)
            nc.vector.tensor_add(T[:].rearrange("p (t d e) -> p t d e", t=TPP, d=5),
                                 M1v, seg_view(0))

            # col interp: U[t, j, i, c] = T(d=j) + fc*(T(d=j+1) - T(d=j))
            Tv = T[:].rearrange("p (t x) -> p t x", t=TPP)
            T0 = Tv[:, :, 0:48].rearrange("p t (d e) -> p t d e", d=4)
            T12 = Tv[:, :, 12:60].rearrange("p t (d e) -> p t d e", d=4)
            D2 = pool.tile([P, TPP * 48], F32)
            D2v = D2[:].rearrange("p (t d e) -> p t d e", t=TPP, d=4)
            nc.vector.tensor_sub(D2v, T12, T0)
            M2 = pool.tile([P, TPP * 48], F32)
            M2v = M2[:].rearrange("p (t d e) -> p t d e", t=TPP, d=4)
            nc.vector.tensor_mul(M2v, D2v,
                                 fc[:].unsqueeze(2).unsqueeze(3).to_broadcast((P, TPP, 4, 12)))
            U = pool.tile([P, TPP * 48], F32)
            nc.vector.tensor_add(U[:].rearrange("p (t d e) -> p t d e", t=TPP, d=4),
                                 M2v, T0)
            if debug:
                nc.sync.dma_start(dbg["U"], U[:])

            # ---- classifier: out[c] = sum(U * W2p[c]) + b2 -----------------
            mW2 = pool.tile([P, NCLS * 1536], F32)
            nc.vector.tensor_mul(mW2[:].rearrange("p (c x) -> p c x", c=NCLS),
                                 W2sb[:].rearrange("p (c x) -> p c x", c=NCLS),
                                 U[:].unsqueeze(1).to_broadcast((P, NCLS, 1536)))
            r2 = pool.tile([P, NCLS], F32)
            nc.vector.reduce_sum(r2[:].unsqueeze(2),
                                 mW2[:].rearrange("p (c x) -> p c x", c=NCLS),
                                 axis=AX.X)
            ppool = ctx.enter_context(tc.tile_pool(name="ps", bufs=1, space="PSUM"))
            ones = pool.tile([P, 1], F32)
            nc.vector.memset(ones[:], 1.0)
            osum = ppool.tile([1, NCLS], F32)
            nc.tensor.matmul(out=osum[:], lhsT=ones[:], rhs=r2[:], start=True, stop=True)
            ofin = pool.tile([1, NCLS], F32)
            nc.vector.tensor_add(ofin[:], osum[:], b2sb[:])
            nc.sync.dma_start(out, ofin[:])

    nc.compile()
    return nc


@functools.lru_cache(maxsize=2)
def _compiled(num_devices: int, svh: int, svw: int, debug: bool = False):
    return build_program(num_devices, svh, svw, debug)


def band_layout(img: np.ndarray) -> np.ndarray:
    """[2048, 2048, 3] -> [511*16384, 3]: 8-row bands at stride 4, [col][row][ch]."""
    sw = np.lib.stride_tricks.sliding_window_view(img, 8, axis=0)  # [2041, 2048, 3, 8]
    sb = sw[::4]                                                   # [511, 2048, 3, 8]
    return np.ascontiguousarray(sb.transpose(0, 1, 3, 2)).reshape(-1, 3)


def permute_w2(W2: np.ndarray) -> np.ndarray:
    """Reorder per-patch (i, j, c) -> (j, i, c) to match the kernel's U layout."""
    return np.ascontiguousarray(
        W2.reshape(NCLS, N, PS, PS, 3).transpose(0, 1, 3, 2, 4)).reshape(NCLS, -1)


def make_in_maps(topview, search_views, W1, b1, W2, b2):
    W1 = np.ascontiguousarray(W1, np.float32)
    b1 = np.ascontiguousarray(b1, np.float32)
    W2p = permute_w2(np.ascontiguousarray(W2, np.float32))
    b2 = np.ascontiguousarray(b2, np.float32)
    return [{
        "tv": np.ascontiguousarray(topview[i], np.float32),
        "svb": band_layout(np.ascontiguousarray(search_views[i], np.float32)),
        "W1": W1, "b1": b1, "W2p": W2p, "b2": b2,
    } for i in range(topview.shape[0])]


def kernel(topview, search_views, W1, b1, W2, b2, svh, svw):
    svh, svw = int(svh), int(svw)
    nc = _compiled(B, svh, svw)
    in_maps = make_in_maps(topview, search_views, W1, b1, W2, b2)
    res = bass_utils.run_bass_kernel_spmd(nc, in_maps, core_ids=list(range(B)))
    return np.concatenate([res.results[i]["out"] for i in range(B)], axis=0)



# revision 6
# speedup vs baseline: 1.3158x; 1.3158x over previous
"""Trainium2 Bass kernel for nn_DZSpecimenClfToy (v4).

Reference computation (per batch item b, B=8, one NeuronCore each):
  1. tv = bilinear_resize(topview[b], (3,64,64) -> (3,4,4))
  2. coords = sigmoid(tv.flat @ W1.T + b1).reshape(N,2)       # N=4096
  3. tl = coords*2043; 5x5x3 bilinear support per patch
  4. out[b] = bilinear_crops.flat @ W2.T + b2                 # [2]

Sharding: data-parallel over batch across 8 cores; weights replicated.

Host re-lays the search view as a cell table svc[r*2048+c] = 16 bf16
(rows r..r+4 of column c, 15 values + pad), so a patch at (r0,c0) is ONE
contiguous 79-bf16 run at cell index r0*2048+c0 (< 2^23: float magic
rounding gives the exact int index; no div/mod needed).

The HW indirect DMA supports one offset per partition per instruction,
so the gather is 32 x [128 offsets] instructions serialized on the
GpSimd Q7 (~1.4us each).  All other compute is pipelined UNDER that
wall: patches are processed in NCHUNK groups - while later groups are
still being gathered, the bf16 bilinear combine + classifier partial
dot-products for earlier groups run on the Vector engine.

Precision: coords matvec in f32 (bf16 shifts patches ~0.1px: fails);
patch data / combine / W2 in bf16 (~5e-3 rel err vs 2e-2 budget).
"""
import functools
from contextlib import ExitStack

import numpy as np
import ml_dtypes

import concourse.bass as bass
import concourse.tile as tile
from concourse import bacc, mybir
import concourse.bass_utils as bass_utils
from concourse.bass import IndirectOffsetOnAxis

F32 = mybir.dt.float32
BF16 = mybir.dt.bfloat16
I32 = mybir.dt.int32
ALU = mybir.AluOpType
ACT = mybir.ActivationFunctionType
AX = mybir.AxisListType

B = 8          # batch == number of cores
H = W = 2048   # search view height/width
N = 4096       # patches per item
PS = 4         # patch size
NCLS = 2       # classes
P = 128        # partitions
TPP = N // P   # patches per partition = 32

R = H - PS     # 2044 rows of 5-row cells (r0 in [0, 2043])
CELL = 16      # bf16 per (row, col) cell: 5 rows x 3 ch + 1 pad
SEG = 79       # gathered bf16 per patch: max offset 4*16+4*3+2 = 78
SEGP = 80      # SBUF stride per patch segment
MAGIC = 8388608.0   # 2**23
NP_BF16 = ml_dtypes.bfloat16

NCHUNK = 4                  # pipeline groups
TPC = TPP // NCHUNK         # patches per partition per group = 8


def build_program(num_devices: int, svh: int, svw: int):
    pad = float(svh - 1 - PS)  # 2043
    assert svh == H and svw == W, (svh, svw)

    nc = bacc.Bacc("TRN2", target_bir_lowering=False, debug=False,
                   enable_asserts=False, num_devices=num_devices)

    tvs = nc.dram_tensor("tvs", [1, 1536], F32, kind="ExternalInput").ap()
    svc = nc.dram_tensor("svc", [R * W, CELL], BF16, kind="ExternalInput").ap()
    w1 = nc.dram_tensor("W1k", [P, 64 * 48], F32, kind="ExternalInput").ap()
    b1 = nc.dram_tensor("b1k", [P, 64], F32, kind="ExternalInput").ap()
    w2 = nc.dram_tensor("W2k", [P, NCLS * 1536], BF16, kind="ExternalInput").ap()
    b2 = nc.dram_tensor("b2k", [1, NCLS], F32, kind="ExternalInput").ap()
    out = nc.dram_tensor("out", [1, NCLS], F32, kind="ExternalOutput").ap()

    JC = 2 * TPC  # coord lanes per group = 16

    with tile.TileContext(nc) as tc:
        with ExitStack() as ctx:
            pool = ctx.enter_context(tc.tile_pool(name="main", bufs=1))

            # ---- input DMAs (all on the sync HWDGE ring, order matters):
            # W1 group-0 slice first (it gates the first matvec), then the
            # broadcast topview load, then the rest.
            W1sb = pool.tile([P, 64 * 48], F32)
            nc.sync.dma_start(W1sb[:, 0:JC * 48], w1[:, 0:JC * 48])
            nc.sync.dma_start(W1sb[:, JC * 48:], w1[:, JC * 48:])
            Ab = pool.tile([P, 1536], F32)
            nc.scalar.dma_start(Ab[:], tvs.to_broadcast((P, 1536)))
            W2sb = pool.tile([P, NCLS * 1536], BF16)
            nc.scalar.dma_start(W2sb[:], w2)
            b1sb = pool.tile([P, 64], F32)
            nc.scalar.dma_start(b1sb[:], b1)
            b2sb = pool.tile([1, NCLS], F32)
            nc.scalar.dma_start(b2sb[:], b2)

            # ---- topview 64x64 -> 4x4 resize, replicated on all partitions
            # (x0.25 folded into W1k on the host)
            V = pool.tile([P, 768], F32)
            A4 = Ab[:].rearrange("p (ck r w) -> p ck r w", ck=12, r=2)
            nc.vector.tensor_add(V[:].rearrange("p (ck w) -> p ck w", ck=12),
                                 A4[:, :, 0, :], A4[:, :, 1, :])
            flatb = pool.tile([P, 48], F32)
            V4 = V[:].rearrange("p (ck g s) -> p ck g s", ck=12, g=4)
            nc.vector.tensor_add(flatb[:].rearrange("p (ck g) -> p ck g", ck=12),
                                 V4[:, :, :, 7], V4[:, :, :, 8])

            # ---- coords + gather index, in NCHUNK j-groups ----------------
            mul1 = pool.tile([P, 64 * 48], F32)
            pre = pool.tile([P, 64], F32)
            sg = pool.tile([P, 64], F32)
            tl = pool.tile([P, 64], F32)
            rnd = pool.tile([P, 64], F32)
            gt = pool.tile([P, 64], F32)
            r0 = pool.tile([P, 64], F32)
            fr64 = pool.tile([P, 64], F32)
            t1 = pool.tile([P, TPP], F32)
            idxm = pool.tile([P, TPP], F32)
            idxi = pool.tile([P, TPP], I32)
            frx = pool.tile([P, TPP * 12], BF16)
            fcx = pool.tile([P, TPP * 12], BF16)

            Schunks = []
            for k in range(NCHUNK):
                Sk = pool.tile([P, TPC * SEGP], BF16, tag=f"S{k}")
                Schunks.append(Sk)

            for k in range(NCHUNK):
                js = slice(k * JC, (k + 1) * JC)
                ts = slice(k * TPC, (k + 1) * TPC)
                m1v = mul1[:, k * JC * 48:(k + 1) * JC * 48] \
                    .rearrange("p (j c) -> p j c", j=JC)
                nc.vector.tensor_mul(
                    m1v, W1sb[:, k * JC * 48:(k + 1) * JC * 48]
                    .rearrange("p (j c) -> p j c", j=JC),
                    flatb[:].unsqueeze(1).to_broadcast((P, JC, 48)))
                nc.vector.reduce_sum(pre[:, js].unsqueeze(2), m1v, axis=AX.X)
                nc.vector.tensor_add(pre[:, js], pre[:, js], b1sb[:, js])
                nc.scalar.activation(sg[:, js], pre[:, js], ACT.Sigmoid)
                nc.vector.tensor_scalar_mul(tl[:, js], sg[:, js], pad)
                nc.vector.tensor_scalar(rnd[:, js], tl[:, js], MAGIC, MAGIC,
                                        op0=ALU.add, op1=ALU.subtract)
                nc.vector.tensor_tensor(gt[:, js], rnd[:, js], tl[:, js],
                                        op=ALU.is_gt)
                nc.vector.tensor_sub(r0[:, js], rnd[:, js], gt[:, js])
                nc.vector.tensor_sub(fr64[:, js], tl[:, js], r0[:, js])
                # idx+MAGIC = (r0*2048 + MAGIC) + c0
                r0v = r0[:, js].rearrange("p (t two) -> p t two", two=2)
                nc.vector.tensor_scalar(t1[:, ts], r0v[:, :, 0], float(W),
                                        MAGIC, op0=ALU.mult, op1=ALU.add)
                nc.vector.tensor_add(idxm[:, ts], t1[:, ts], r0v[:, :, 1])
                nc.vector.tensor_single_scalar(idxi[:, ts],
                                               idxm[:, ts].bitcast(I32),
                                               0x007FFFFF, op=ALU.bitwise_and)
                fr4 = fr64[:, js].rearrange("p (t two) -> p t two", two=2)
                nc.vector.tensor_scalar_mul(
                    frx[:, k * TPC * 12:(k + 1) * TPC * 12]
                    .rearrange("p (t e) -> p t e", t=TPC),
                    fr4[:, :, 0:1].to_broadcast((P, TPC, 12)), 1.0)
                nc.vector.tensor_scalar_mul(
                    fcx[:, k * TPC * 12:(k + 1) * TPC * 12]
                    .rearrange("p (t e) -> p t e", t=TPC),
                    fr4[:, :, 1:2].to_broadcast((P, TPC, 12)), 1.0)

                # gathers for this group: TPC instructions, 128 offsets each
                S = Schunks[k]
                for t in range(TPC):
                    tg = k * TPC + t
                    nc.gpsimd.indirect_dma_start(
                        out=S[:, t * SEGP:t * SEGP + SEG],
                        out_offset=None,
                        in_=svc,
                        in_offset=IndirectOffsetOnAxis(
                            ap=idxi[:, tg:tg + 1], axis=0),
                    )

            # ---- per-group combine + classifier (overlaps later gathers) --
            D1 = pool.tile([P, TPC * 60], BF16)
            M1 = pool.tile([P, TPC * 60], BF16)
            T = pool.tile([P, TPC * 60], BF16)
            D2 = pool.tile([P, TPC * 48], BF16)
            M2 = pool.tile([P, TPC * 48], BF16)
            U = pool.tile([P, TPC * 48], BF16)
            Pm = pool.tile([P, TPC * 48], BF16)
            r2all = pool.tile([P, NCLS * NCHUNK], F32)
            ppool = ctx.enter_context(tc.tile_pool(name="ps", bufs=1,
                                                   space="PSUM"))
            ones = pool.tile([P, 1], F32)
            nc.vector.memset(ones[:], 1.0)

            for k in range(NCHUNK):
                S = Schunks[k]
                Sc = S[:].rearrange("p (t d e) -> p t d e", t=TPC, d=5)
                S0 = Sc[:, :, :, 0:12]
                S1 = Sc[:, :, :, 3:15]
                frb = frx[:, k * TPC * 12:(k + 1) * TPC * 12] \
                    .rearrange("p (t e) -> p t e", t=TPC).unsqueeze(2) \
                    .to_broadcast((P, TPC, 5, 12))
                fcb = fcx[:, k * TPC * 12:(k + 1) * TPC * 12] \
                    .rearrange("p (t e) -> p t e", t=TPC).unsqueeze(2) \
                    .to_broadcast((P, TPC, 4, 12))
                D1v = D1[:].rearrange("p (t d e) -> p t d e", t=TPC, d=5)
                nc.vector.tensor_sub(D1v, S1, S0)
                M1v = M1[:].rearrange("p (t d e) -> p t d e", t=TPC, d=5)
                nc.vector.tensor_mul(M1v, D1v, frb)
                Tv = T[:].rearrange("p (t d e) -> p t d e", t=TPC, d=5)
                nc.vector.tensor_add(Tv, M1v, S0)
                T0 = Tv[:, :, 0:4, :]
                T1 = Tv[:, :, 1:5, :]
                D2v = D2[:].rearrange("p (t d e) -> p t d e", t=TPC, d=4)
                nc.vector.tensor_sub(D2v, T1, T0)
                M2v = M2[:].rearrange("p (t d e) -> p t d e", t=TPC, d=4)
                nc.vector.tensor_mul(M2v, D2v, fcb)
                nc.vector.tensor_add(
                    U[:].rearrange("p (t d e) -> p t d e", t=TPC, d=4),
                    M2v, T0)
                for c in range(NCLS):
                    nc.vector.tensor_mul(
                        Pm[:], U[:],
                        W2sb[:, c * 1536 + k * TPC * 48:
                             c * 1536 + (k + 1) * TPC * 48])
                    nc.vector.reduce_sum(
                        r2all[:, k * NCLS + c:k * NCLS + c + 1].unsqueeze(2),
                        Pm[:].unsqueeze(1), axis=AX.X)

            # ---- final: sum group partials, partition-reduce, bias, store -
            r2 = pool.tile([P, NCLS], F32)
            r2v = r2all[:].rearrange("p (k c) -> p k c", k=NCHUNK)
            nc.vector.reduce_sum(r2[:].unsqueeze(1),
                                 r2v.rearrange("p k c -> p c k"), axis=AX.X)
            osum = ppool.tile([1, NCLS], F32)
            nc.tensor.matmul(out=osum[:], lhsT=ones[:], rhs=r2[:],
                             start=True, stop=True)
            ofin = pool.tile([1, NCLS], F32)
            nc.vector.tensor_add(ofin[:], osum[:], b2sb[:])
            nc.sync.dma_start(out, ofin[:])

    nc.compile()
    return nc


@functools.lru_cache(maxsize=2)
def _compiled(num_devices: int, svh: int, svw: int):
    return build_program(num_devices, svh, svw)


def cell_layout(img: np.ndarray) -> np.ndarray:
    """[2048, 2048, 3] f32 -> [2044*2048, 16] bf16 cell table."""
    sw = np.lib.stride_tricks.sliding_window_view(img, 5, axis=0)  # [2044,2048,3,5]
    cells = sw.transpose(0, 1, 3, 2).reshape(R, W, 15)             # (row, ch)
    buf = np.zeros((R, W, CELL), dtype=NP_BF16)
    buf[:, :, :15] = cells.astype(NP_BF16)
    return buf.reshape(R * W, CELL)


def permute_w2(W2: np.ndarray) -> np.ndarray:
    """(n, i, j, c) -> (n, j, i, c), then [p, (cls, t*48+x)] bf16."""
    w = W2.reshape(NCLS, N, PS, PS, 3).transpose(0, 1, 3, 2, 4)
    w = w.reshape(NCLS, P, TPP * 48).transpose(1, 0, 2)
    return np.ascontiguousarray(w.reshape(P, NCLS * 1536)).astype(NP_BF16)


def select_tv(tv: np.ndarray) -> np.ndarray:
    """[3,64,64] -> [1, 1536] rows {7,8},{23,24},{39,40},{55,56}."""
    sel = tv[:, (7, 8, 23, 24, 39, 40, 55, 56), :]
    return np.ascontiguousarray(sel.reshape(3, 4, 2, 64)).reshape(1, 1536)


def make_in_maps(topview, search_views, W1, b1, W2, b2):
    W1k = np.ascontiguousarray(
        (0.25 * np.asarray(W1, np.float32)).reshape(P, 64 * 48))
    b1k = np.ascontiguousarray(np.asarray(b1, np.float32).reshape(P, 64))
    W2k = permute_w2(np.ascontiguousarray(W2, np.float32))
    b2k = np.ascontiguousarray(np.asarray(b2, np.float32).reshape(1, NCLS))
    return [{
        "tvs": select_tv(np.ascontiguousarray(topview[i], np.float32)),
        "svc": cell_layout(np.ascontiguousarray(search_views[i], np.float32)),
        "W1k": W1k, "b1k": b1k, "W2k": W2k, "b2k": b2k,
    } for i in range(topview.shape[0])]


def kernel(topview, search_views, W1, b1, W2, b2, svh, svw):
    svh, svw = int(svh), int(svw)
    nc = _compiled(B, svh, svw)
    in_maps = make_in_maps(topview, search_views, W1, b1, W2, b2)
    res = bass_utils.run_bass_kernel_spmd(nc, in_maps, core_ids=list(range(B)))
    return np.concatenate([res.results[i]["out"] for i in range(B)], axis=0)
